# revision 19
# baseline (speedup 1.0000x reference)
"""Trainium2 Bass kernel for nn_BAE_k_deform_k_bae_corse2fine_affine.

Strategy:
  - Host (numpy, fp32): PointNet++ encoder (FPS / ball-query / small SA MLPs,
    ~2 GFLOP total), the z-embedding chains, pose MLP, and folding of all
    z-dependent biases.  These are tiny and/or inherently sequential.
  - Device (8 NeuronCores): the per-query-point MLP stacks (deform MLP,
    per-part BAE MLP, coarse2fine MLP; ~248 GFLOP), sharded over the
    query-point axis (1024 points per core per batch).  Weights replicated.
  - All first-layer (K=3) biases are folded into the matmul via an extra
    ones-row on the moving operand; mid-layer biases ride the activation op.
"""

import numpy as np
from contextlib import ExitStack

# ---------------------------------------------------------------- constants
T, PD, ZD, GF = 4, 64, 256, 256
B, NQ, NCORES = 4, 8192, 8
QC = NQ // NCORES            # query points per core (per batch)
NT = 512                     # device point-tile (matmul moving free dim)
NTILES = QC // NT
BN_SCALE = np.float32(1.0 / np.sqrt(1.0 + 1e-5))
SA_CFG = [
    (1024, 0.1, 32, 6, [32, 32, 64]),
    (256, 0.2, 32, 67, [64, 64, 128]),
    (64, 0.4, 32, 131, [128, 128, 256]),
    (16, 0.8, 32, 259, [256, 256, ZD]),
]

MM_DTYPE = "f32r"            # 'f32r' (fast fp32 PE mode) or 'f32'

# --- schedule tuning knobs (read by build_program) ---
CFG = {
    "pair_dl1": True,    # paired evac for deform L1
    "pair_bl1": True,    # paired evac for bae L1
    "pair_fl1": True,    # paired evac for fine L1
    "ps2_bufs": 2,       # [128,1024] psum pool (2 banks each)
    "ps_bufs": 2,        # [128,512] psum pool
    "pss_bufs": 1,       # small psum pool
    "g1f1_bufs": 12,
    "pf2_bufs": 9,
    "dh1_bufs": 3,
    "dh2_bufs": 4,
    "xg_bufs": 5,
    "gs_bufs": 2,
    "deform_first": True,
    "bae_sync": 2,       # 0: chained parts; 2: pair-synced; 4: fully layer-synced
}

# wd23 region columns: dw2[i] @ i*512 (k*256+m*128), dw3[i] @ 2048+i*8 (k*4)
WD23_COLS = 2080
# wb2: bw2[i] @ i*2048 (k*256+m*128) [bf16]; wb3: bw3[i] @ i*2 (k) [f32r]
WB2_COLS = 8192
WB3_COLS = 8
WF1_COLS = 8192              # k*1024 + m*128
WF2_COLS = 8192
WF3_COLS = 8                 # k
WTS3_COLS = 40 * 128         # per-b: 40 tiles of [4,128] side by side (base partition 0)
WF1C_COLS = 8 * 128          # fine L1 coord tiles (batch-independent)
BIAS_COLS = 41

# bias column map
def _bc_db2(i, m): return i * 2 + m          # deform L2 bias   [128]
def _bc_db3(i): return 8 + i                 # deform L3 bias   [4] (row3=0)
def _bc_bb2(i, m): return 12 + i * 2 + m     # bae L2 bias      [128]
def _bc_bb3(i): return 20 + i                # bae L3 bias      [1]
def _bc_fb2(m): return 24 + m                # fine L2 bias     [128]
def _bc_fb3(): return 32                     # fine L3 bias     [1]
def _bc_scal(i): return 33 + i               # affine scale     [4] (row3=0)
def _bc_trans(i): return 37 + i              # affine trans     [4] (row3=0)

def _wts3_tile_dw1(i, m): return i * 2 + m          # per-b tile index 0..7
def _wts3_tile_bw1(i, m): return 8 + i * 8 + m       # per-b tile index 8..39


# ---------------------------------------------------------------- host math
def _gather(a, idx):
    # a [B,N,C], idx [B,...] -> [B,...,C]
    return a[np.arange(a.shape[0])[:, None], idx.reshape(idx.shape[0], -1)].reshape(
        idx.shape + (a.shape[-1],)
    )


def _fps(xyz, npoint):
    b, n, _ = xyz.shape
    dist = np.full((b, n), 1e10, np.float32)
    far = np.zeros((b,), np.int32)
    idx = np.empty((npoint, b), np.int32)
    ar = np.arange(b)
    for s in range(npoint):
        idx[s] = far
        c = xyz[ar, far][:, None, :]
        d = ((xyz - c) ** 2).sum(-1)
        dist = np.minimum(dist, d)
        far = np.argmax(dist, -1).astype(np.int32)
    return idx.T


def _query_ball(radius, nsample, xyz, new_xyz):
    b, n, _ = xyz.shape
    s = new_xyz.shape[1]
    out = np.empty((b, s, nsample), np.int32)
    r2 = np.float32(radius) * np.float32(radius)
    arn = np.arange(n, dtype=np.int32)
    chunk = max(1, (1 << 22) // n)
    for c0 in range(0, s, chunk):
        c1 = min(s, c0 + chunk)
        sqr = ((new_xyz[:, c0:c1, None, :] - xyz[:, None, :, :]) ** 2).sum(-1)
        idx = np.where(sqr > r2, np.int32(n), arn[None, None, :])
        idx = np.sort(idx, axis=-1)[:, :, :nsample]
        first = idx[:, :, :1]
        out[:, c0:c1] = np.where(idx == n, first, idx)
    return out


def _sa(xyz, pts, npoint, radius, nsample, convs):
    fidx = _fps(xyz, npoint)
    new_xyz = _gather(xyz, fidx)
    gidx = _query_ball(radius, nsample, xyz, new_xyz)
    x = np.concatenate(
        [_gather(xyz, gidx) - new_xyz[:, :, None, :], _gather(pts, gidx)], -1
    )
    for w, b_ in convs:
        x = np.maximum((x @ w + b_) * BN_SCALE, 0.0).astype(np.float32)
    return new_xyz, x.max(axis=2)


def _host_precompute(fps_point, point_coord, params):
    enc, de, gen = params["enc"], params["deform"], params["gen"]
    xyz = pts = np.asarray(fps_point, np.float32)
    for i, (npoint, r, k, _, _) in enumerate(SA_CFG):
        convs = [(np.asarray(w, np.float32), np.asarray(b_, np.float32))
                 for (w, b_) in enc["sa%d" % i]]
        xyz, pts = _sa(xyz, pts, npoint, r, k, convs)
    z = np.einsum("bnc,n->bc", pts, np.asarray(enc["conv0_w"], np.float32)) + np.asarray(
        enc["conv0_b"], np.float32
    )  # [B, ZD]

    def chain(z0, layers):
        e = z0
        for w, b_ in layers:
            e = e @ np.asarray(w, np.float32) + np.asarray(b_, np.float32)
        return e

    ez_d = chain(z, de["embed1"])          # [B, T*PD]
    ez_g = chain(z, gen["embed1"])         # [B, T*PD]
    pcodes = np.asarray(de["part_codes"], np.float32)          # [T, PD]
    ep = pcodes @ np.asarray(gen["embed2_w"], np.float32) + np.asarray(
        gen["embed2_b"], np.float32
    )                                                           # [T, PD]
    (pw1, pb1), (pw2, pb2), (pw3, pb3) = [
        (np.asarray(w, np.float32), np.asarray(b_, np.float32)) for (w, b_) in gen["pose"]
    ]
    pp = np.maximum(ep @ pw1 + pb1, 0.0)
    pp = np.maximum(pp @ pw2 + pb2, 0.0)
    pp = pp @ pw3 + pb3                                         # [T, 6]
    trans = np.tanh(pp[:, :3]).astype(np.float32)               # [T, 3]
    scal = (1.0 / (1.0 + np.exp(-pp[:, 3:]))).astype(np.float32)

    e3w = np.asarray(gen["embed3_w"], np.float32)
    e3b = np.asarray(gen["embed3_b"], np.float32)
    bias_d = np.empty((B, T, 256), np.float32)
    bias_g = np.empty((B, T, GF * 4), np.float32)
    for i in range(T):
        zdef = ez_d[:, i * PD:(i + 1) * PD] + pcodes[i]         # [B, PD]
        w1, b1 = de["w1"][i], de["b1"][i]
        bias_d[:, i] = zdef @ np.asarray(w1, np.float32)[3:] + np.asarray(b1, np.float32)
        zf = (ez_g[:, i * PD:(i + 1) * PD] + ep[i]) @ e3w + e3b  # [B, PD]
        bw1, bb1 = gen["bae_w1"][i], gen["bae_b1"][i]
        bias_g[:, i] = zf @ np.asarray(bw1, np.float32)[3:] + np.asarray(bb1, np.float32)
    return bias_d, bias_g, scal, trans


def _pack_ktiles(w):
    k, m = w.shape
    nk = k // 128
    return np.ascontiguousarray(
        w.reshape(nk, 128, m).transpose(1, 0, 2).reshape(128, nk * m)
    )


def _pack_device_inputs(point_coord, params, bias_d, bias_g, scal, trans):
    de, gen = params["deform"], params["gen"]
    f32 = np.float32

    import ml_dtypes
    bf16 = ml_dtypes.bfloat16
    wd23 = np.zeros((128, WD23_COLS), f32)
    wb2 = np.zeros((128, WB2_COLS), bf16)
    wb3 = np.zeros((128, WB3_COLS), f32)
    for i in range(T):
        wd23[:, i * 512:(i + 1) * 512] = _pack_ktiles(np.asarray(de["w2"][i], f32))
        w3 = np.zeros((256, 4), f32)
        w3[:, :3] = np.asarray(de["w3"][i], f32)
        wd23[:, 2048 + i * 8:2048 + (i + 1) * 8] = _pack_ktiles(w3)
        wb2[:, i * 2048:(i + 1) * 2048] = _pack_ktiles(
            np.asarray(gen["bae_w2"][i], f32)).astype(bf16)
        wb3[:, i * 2:(i + 1) * 2] = _pack_ktiles(np.asarray(gen["bae_w3"][i], f32))
    fw1 = np.asarray(gen["fine_w1"], f32)          # [1027, 1024]
    wf1 = _pack_ktiles(fw1[3:])
    wf2 = _pack_ktiles(np.asarray(gen["fine_w2"], f32)).astype(bf16)
    wf3 = _pack_ktiles(np.asarray(gen["fine_w3"], f32))

    wts3 = np.zeros((B, 4, WTS3_COLS), f32)

    def put3(b, tidx, rows3, brow):
        c = 128 * tidx
        wts3[b, 0:3, c:c + 128] = rows3
        wts3[b, 3, c:c + 128] = brow

    for b in range(B):
        for i in range(T):
            w1 = np.asarray(de["w1"][i], f32)
            for m in range(2):
                put3(b, _wts3_tile_dw1(i, m), w1[0:3, m * 128:(m + 1) * 128],
                     bias_d[b, i, m * 128:(m + 1) * 128])
            bw1 = np.asarray(gen["bae_w1"][i], f32)
            for m in range(8):
                put3(b, _wts3_tile_bw1(i, m), bw1[0:3, m * 128:(m + 1) * 128],
                     bias_g[b, i, m * 128:(m + 1) * 128])
    wf1c = np.zeros((4, WF1C_COLS), f32)
    fb1 = np.asarray(gen["fine_b1"], f32)
    for m in range(8):
        wf1c[0:3, m * 128:(m + 1) * 128] = fw1[0:3, m * 128:(m + 1) * 128]
        wf1c[3, m * 128:(m + 1) * 128] = fb1[m * 128:(m + 1) * 128]

    bias = np.zeros((128, BIAS_COLS), f32)
    for i in range(T):
        db2 = np.asarray(de["b2"][i], f32)
        bb2 = np.asarray(gen["bae_b2"][i], f32)
        for m in range(2):
            bias[:, _bc_db2(i, m)] = db2[m * 128:(m + 1) * 128]
            bias[:, _bc_bb2(i, m)] = bb2[m * 128:(m + 1) * 128]
        bias[0:3, _bc_db3(i)] = np.asarray(de["b3"][i], f32)
        bias[0, _bc_bb3(i)] = np.asarray(gen["bae_b3"][i], f32)[0]
        bias[0:3, _bc_scal(i)] = scal[i]
        bias[0:3, _bc_trans(i)] = trans[i]
    fb2 = np.asarray(gen["fine_b2"], f32)
    for m in range(8):
        bias[:, _bc_fb2(m)] = fb2[m * 128:(m + 1) * 128]
    bias[0, _bc_fb3()] = np.asarray(gen["fine_b3"], f32)[0]

    # per-core transposed coords with ones row
    pc_full = np.ones((B, 4, NQ), f32)
    pc_full[:, 0:3, :] = np.asarray(point_coord, f32).transpose(0, 2, 1)
    pcs = [np.ascontiguousarray(pc_full[:, :, c * QC:(c + 1) * QC]) for c in range(NCORES)]

    shared = {"wd23": wd23, "wb2": wb2, "wb3": wb3, "wf1": wf1, "wf2": wf2,
              "wf3": wf3, "wts3": wts3, "wf1c": wf1c, "bias": bias}
    return pcs, shared


# ---------------------------------------------------------------- device IR
def build_program():
    import concourse.bass as bass
    import concourse.mybir as mybir
    import concourse.tile as tile
    from concourse import bacc
    from contextlib import ExitStack

    f32 = mybir.dt.float32
    AF = mybir.ActivationFunctionType
    OP = mybir.AluOpType
    # float32r tiles are produced rounded, letting the PE run its fast
    # single-pass fp32 mode (1 cycle/row at N>=256).
    fr = mybir.dt.float32r if MM_DTYPE == "f32r" else f32

    nc = bacc.Bacc("TRN2", target_bir_lowering=False, debug=False)
    pc_d = nc.dram_tensor("pc", [B, 4, QC], fr, kind="ExternalInput").ap()
    wd23_d = nc.dram_tensor("wd23", [128, WD23_COLS], fr, kind="ExternalInput").ap()
    bf = mybir.dt.bfloat16
    wb2_d = nc.dram_tensor("wb2", [128, WB2_COLS], bf, kind="ExternalInput").ap()
    wb3_d = nc.dram_tensor("wb3", [128, WB3_COLS], fr, kind="ExternalInput").ap()
    wf1_d = nc.dram_tensor("wf1", [128, WF1_COLS], fr, kind="ExternalInput").ap()
    wf2_d = nc.dram_tensor("wf2", [128, WF2_COLS], bf, kind="ExternalInput").ap()
    wf3_d = nc.dram_tensor("wf3", [128, WF3_COLS], fr, kind="ExternalInput").ap()
    wts3_d = nc.dram_tensor("wts3", [B, 4, WTS3_COLS], fr, kind="ExternalInput").ap()
    wf1c_d = nc.dram_tensor("wf1c", [4, WF1C_COLS], fr, kind="ExternalInput").ap()
    bias_d_ = nc.dram_tensor("bias", [128, BIAS_COLS], f32, kind="ExternalInput").ap()
    gs_d = nc.dram_tensor("gs", [B, T, QC], f32, kind="ExternalOutput").ap()
    fo_d = nc.dram_tensor("fo", [B, QC], f32, kind="ExternalOutput").ap()

    N2 = 2 * NT

    with tile.TileContext(nc) as tc:
        with ExitStack() as ctx:
            wp = ctx.enter_context(tc.tile_pool(name="weights", bufs=1))
            xdp = ctx.enter_context(tc.tile_pool(name="xd", bufs=2))
            affp = ctx.enter_context(tc.tile_pool(name="aff", bufs=2))
            xgp = ctx.enter_context(tc.tile_pool(name="xg", bufs=CFG["xg_bufs"]))
            dh1p = ctx.enter_context(tc.tile_pool(name="dh1", bufs=CFG["dh1_bufs"]))
            dh2p = ctx.enter_context(tc.tile_pool(name="dh2", bufs=CFG["dh2_bufs"]))
            g1f1p = ctx.enter_context(tc.tile_pool(name="g1f1", bufs=CFG["g1f1_bufs"]))
            pf2p = ctx.enter_context(tc.tile_pool(name="pf2", bufs=CFG["pf2_bufs"]))
            gsp = ctx.enter_context(tc.tile_pool(name="gsst", bufs=CFG["gs_bufs"]))
            fsbp = ctx.enter_context(tc.tile_pool(name="fsb", bufs=2))
            psp2 = ctx.enter_context(tc.tile_pool(name="ps2", bufs=CFG["ps2_bufs"], space="PSUM"))
            psp = ctx.enter_context(tc.tile_pool(name="ps", bufs=CFG["ps_bufs"], space="PSUM"))
            pssp = ctx.enter_context(tc.tile_pool(name="pss", bufs=CFG["pss_bufs"], space="PSUM"))

            bias = wp.tile([128, BIAS_COLS], f32, tag="bias")
            nc.sync.dma_start(bias[:], bias_d_[:])
            # big weight blocks are DMA'd lazily (below) so the first combo's
            # staging/coord loads aren't queued behind ~7MB of weights
            wd23 = wp.tile([128, WD23_COLS], fr, tag="wd23")
            wb2 = wp.tile([128, WB2_COLS], bf, tag="wb2")
            wb3 = wp.tile([128, WB3_COLS], fr, tag="wb3")
            wf1 = wp.tile([128, WF1_COLS], fr, tag="wf1")
            wf2 = wp.tile([128, WF2_COLS], bf, tag="wf2")
            wf3 = wp.tile([128, WF3_COLS], fr, tag="wf3")
            wf1c = wp.tile([4, WF1C_COLS], fr, tag="wf1c")
            cm1 = wp.tile([4, NT], f32, tag="cm1")
            nc.gpsimd.memset(cm1[:], -1.0)
            cp1 = wp.tile([4, NT], f32, tag="cp1")
            nc.gpsimd.memset(cp1[:], 1.0)

            def w3ap(stage, tidx):
                return stage[0:4, 128 * tidx:128 * tidx + 128]

            def bcol(c, rows=128):
                return bias[0:rows, c:c + 1]

            def prelu(dst, ps, bias_ap=0.0):
                nc.scalar.activation(dst, ps, AF.Prelu, bias=bias_ap, alpha=0.01)

            for b in range(B):
                wts3 = wp.tile([4, WTS3_COLS], fr, tag="wts3", name="wts3st", bufs=1)
                nc.sync.dma_start(wts3[:], wts3_d[b])
                for t in range(NTILES):
                    sl = slice(t * NT, (t + 1) * NT)
                    xd = xdp.tile([4, NT], fr, tag="xd")
                    nc.sync.dma_start(xd[:], pc_d[b, :, sl])
                    if b == 0 and t == 0:
                        nc.sync.dma_start(wd23[:], wd23_d[:])
                        nc.sync.dma_start(wb2[:], wb2_d[:])
                        nc.sync.dma_start(wb3[:], wb3_d[:])
                    pfe = [pf2p.tile([128, NT], fr, tag="pf2", name="pfe")
                           for _ in range(8)]
                    def deform_chain(i):
                        # deform L1: K=4 (coords+ones), M=256
                        h1 = dh1p.tile([128, N2], fr, tag="dh1", name="dh1t")
                        if CFG["pair_dl1"]:
                            pp = psp2.tile([128, N2], f32, tag="ps2", name="pdl1")
                            for m in range(2):
                                nc.tensor.matmul(pp[:, m * NT:(m + 1) * NT],
                                                 w3ap(wts3, _wts3_tile_dw1(i, m)),
                                                 xd[:], start=True, stop=True)
                            prelu(h1[:], pp[:])
                        else:
                            for m in range(2):
                                ps = psp.tile([128, NT], f32, tag="ps", name="pdl1")
                                nc.tensor.matmul(ps[:], w3ap(wts3, _wts3_tile_dw1(i, m)),
                                                 xd[:], start=True, stop=True)
                                prelu(h1[:, m * NT:(m + 1) * NT], ps[:])
                        # deform L2: K=256, M=256
                        h2 = []
                        for m in range(2):
                            ps = psp.tile([128, NT], f32, tag="ps", name="pdl2")
                            for k in range(2):
                                w = wd23[:, i * 512 + k * 256 + m * 128:
                                         i * 512 + k * 256 + m * 128 + 128]
                                nc.tensor.matmul(ps[:], w, h1[:, k * NT:(k + 1) * NT],
                                                 start=(k == 0), stop=(k == 1))
                            h = dh2p.tile([128, NT], fr, tag="dh2", name="dh2t")
                            prelu(h[:], ps[:], bcol(_bc_db2(i, m)))
                            h2.append(h)
                        # deform L3 (M=4 padded; row3 stays 0) + tanh + affine
                        ps3 = pssp.tile([4, NT], f32, tag="pss", name="pdl3")
                        for k in range(2):
                            w = wd23[:, 2048 + i * 8 + k * 4:2048 + i * 8 + k * 4 + 4]
                            nc.tensor.matmul(ps3[:], w, h2[k][:],
                                             start=(k == 0), stop=(k == 1))
                        t3 = affp.tile([4, NT], f32, tag="aff", name="t3t")
                        nc.scalar.activation(t3[:], ps3[:], AF.Tanh,
                                             bias=bcol(_bc_db3(i), 4))
                        nc.scalar.activation(t3[:], t3[:], AF.Identity,
                                             bias=bcol(_bc_trans(i), 4),
                                             scale=bcol(_bc_scal(i), 4))
                        nc.vector.tensor_tensor(t3[:], t3[:], cm1[:], OP.max)
                        nc.vector.tensor_tensor(t3[:], t3[:], cp1[:], OP.min)
                        xg = xgp.tile([4, NT], fr, tag="xg")
                        nc.vector.tensor_tensor(xg[:], t3[:], xd[:], OP.add)
                        return xg

                    def bae_l1(i, xg):
                        # bae L1: K=4, M=1024 as 4 [128,1024] outputs
                        g1 = []
                        for mp in range(4):
                            g = g1f1p.tile([128, N2], bf, tag="g1f1", name="g1t")
                            if CFG["pair_bl1"]:
                                pp = psp2.tile([128, N2], f32, tag="ps2", name="pbl1")
                                for h in range(2):
                                    nc.tensor.matmul(
                                        pp[:, h * NT:(h + 1) * NT],
                                        w3ap(wts3, _wts3_tile_bw1(i, 2 * mp + h)),
                                        xg[:], start=True, stop=True)
                                prelu(g[:], pp[:])
                            else:
                                for h in range(2):
                                    ps = psp.tile([128, NT], f32, tag="ps", name="pbl1")
                                    nc.tensor.matmul(
                                        ps[:], w3ap(wts3, _wts3_tile_bw1(i, 2 * mp + h)),
                                        xg[:], start=True, stop=True)
                                    prelu(g[:, h * NT:(h + 1) * NT], ps[:])
                            g1.append(g)
                        return g1

                    def bae_l23(i, g1):
                        # bae L2: K=1024, M=256
                        for m in range(2):
                            ps = psp.tile([128, NT], f32, tag="ps", name="pbl2")
                            for k in range(8):
                                w = wb2[:, i * 2048 + k * 256 + m * 128:
                                        i * 2048 + k * 256 + m * 128 + 128]
                                nc.tensor.matmul(ps[:], w,
                                                 g1[k // 2][:, (k % 2) * NT:
                                                            (k % 2) * NT + NT],
                                                 start=(k == 0), stop=(k == 7))
                            prelu(pfe[i * 2 + m][:], ps[:], bcol(_bc_bb2(i, m)))
                        # bae L3 logit + sigmoid
                        psg = pssp.tile([1, NT], f32, tag="pss", name="pbl3")
                        for k in range(2):
                            w = wb3[:, i * 2 + k:i * 2 + k + 1]
                            nc.tensor.matmul(psg[:], w, pfe[i * 2 + k][:],
                                             start=(k == 0), stop=(k == 1))
                        gs_sb = gsp.tile([1, NT], f32, tag="gs", name="gs_sb")
                        nc.scalar.activation(gs_sb[:], psg[:], AF.Sigmoid,
                                             bias=bcol(_bc_bb3(i), 1))
                        nc.sync.dma_start(gs_d[b, i:i + 1, sl], gs_sb[:])

                    def bae_chain(i, xg):
                        bae_l23(i, bae_l1(i, xg))

                    if b == 0 and t == 0:
                        nc.sync.dma_start(wf1c[:], wf1c_d[:])
                        nc.sync.dma_start(wf1[:], wf1_d[:])
                        nc.sync.dma_start(wf2[:], wf2_d[:])
                        nc.sync.dma_start(wf3[:], wf3_d[:])
                    if CFG["deform_first"]:
                        xgs = [deform_chain(i) for i in range(T)]
                        sync = CFG["bae_sync"]
                        if sync == 0:
                            for i in range(T):
                                bae_chain(i, xgs[i])
                        else:
                            for p0 in range(0, T, sync):
                                grp = range(p0, p0 + sync)
                                g1s = {i: bae_l1(i, xgs[i]) for i in grp}
                                for i in grp:
                                    bae_l23(i, g1s[i])
                    else:
                        for i in range(T):
                            bae_chain(i, deform_chain(i))
                    # fine L1: K=1027 (coords+ones fold + pfe), M=1024, 4 pairs
                    f1 = []
                    for mp in range(4):
                        g = g1f1p.tile([128, N2], bf, tag="g1f1", name="f1t")
                        if CFG["pair_fl1"]:
                            pp = psp2.tile([128, N2], f32, tag="ps2", name="pfl1")
                            for h in range(2):
                                m = 2 * mp + h
                                dst = pp[:, h * NT:(h + 1) * NT]
                                nc.tensor.matmul(dst, w3ap(wf1c, m), xd[:],
                                                 start=True, stop=False)
                                for k in range(8):
                                    w = wf1[:, k * 1024 + m * 128:k * 1024 + m * 128 + 128]
                                    nc.tensor.matmul(dst, w, pfe[k][:],
                                                     start=False, stop=(k == 7))
                            prelu(g[:], pp[:])
                        else:
                            for h in range(2):
                                m = 2 * mp + h
                                ps = psp.tile([128, NT], f32, tag="ps", name="pfl1")
                                nc.tensor.matmul(ps[:], w3ap(wf1c, m), xd[:],
                                                 start=True, stop=False)
                                for k in range(8):
                                    w = wf1[:, k * 1024 + m * 128:k * 1024 + m * 128 + 128]
                                    nc.tensor.matmul(ps[:], w, pfe[k][:],
                                                     start=False, stop=(k == 7))
                                prelu(g[:, h * NT:(h + 1) * NT], ps[:])
                        f1.append(g)
                    # fine L2: K=1024, M=1024
                    f2 = []
                    for m in range(8):
                        ps = psp.tile([128, NT], f32, tag="ps", name="pfl2")
                        for k in range(8):
                            w = wf2[:, k * 1024 + m * 128:k * 1024 + m * 128 + 128]
                            nc.tensor.matmul(ps[:], w,
                                             f1[k // 2][:, (k % 2) * NT:(k % 2) * NT + NT],
                                             start=(k == 0), stop=(k == 7))
                        g = pf2p.tile([128, NT], fr, tag="pf2", name="f2t")
                        prelu(g[:], ps[:], bcol(_bc_fb2(m)))
                        f2.append(g)
                    # fine L3 + sigmoid
                    psf = pssp.tile([1, NT], f32, tag="pss", name="pfl3")
                    for k in range(8):
                        nc.tensor.matmul(psf[:], wf3[:, k:k + 1], f2[k][:],
                                         start=(k == 0), stop=(k == 7))
                    fsb = fsbp.tile([1, NT], f32, tag="fsb")
                    nc.scalar.activation(fsb[:], psf[:], AF.Sigmoid,
                                         bias=bcol(_bc_fb3(), 1))
                    nc.sync.dma_start(fo_d[b:b + 1, sl], fsb[:])
    nc.compile()
    return nc


def build_noop_program():
    """Same I/O signature, trivial body — times the dispatch/transfer floor."""
    import concourse.mybir as mybir
    import concourse.tile as tile
    from concourse import bacc
    from contextlib import ExitStack

    f32 = mybir.dt.float32
    fr = mybir.dt.float32r if MM_DTYPE == "f32r" else f32
    nc = bacc.Bacc("TRN2", target_bir_lowering=False, debug=False)
    pc_d = nc.dram_tensor("pc", [B, 4, QC], fr, kind="ExternalInput").ap()
    nc.dram_tensor("wd23", [128, WD23_COLS], fr, kind="ExternalInput")
    nc.dram_tensor("wb23", [128, WB23_COLS], fr, kind="ExternalInput")
    nc.dram_tensor("wf1", [128, WF1_COLS], fr, kind="ExternalInput")
    nc.dram_tensor("wf2", [128, WF2_COLS], fr, kind="ExternalInput")
    nc.dram_tensor("wf3", [128, WF3_COLS], fr, kind="ExternalInput")
    nc.dram_tensor("wts3", [B, 4, WTS3_COLS], fr, kind="ExternalInput")
    nc.dram_tensor("wf1c", [4, WF1C_COLS], fr, kind="ExternalInput")
    nc.dram_tensor("bias", [128, BIAS_COLS], f32, kind="ExternalInput")
    gs_d = nc.dram_tensor("gs", [B, T, QC], f32, kind="ExternalOutput").ap()
    fo_d = nc.dram_tensor("fo", [B, QC], f32, kind="ExternalOutput").ap()
    with tile.TileContext(nc) as tc:
        with ExitStack() as ctx:
            p = ctx.enter_context(tc.tile_pool(name="p", bufs=1))
            t = p.tile([4, QC], fr, tag="t")
            nc.sync.dma_start(t[:], pc_d[0])
            t2 = p.tile([4, QC], f32, tag="t2")
            nc.vector.tensor_copy(t2[:], t[:])
            for b in range(B):
                for i in range(T):
                    nc.sync.dma_start(gs_d[b, i:i + 1, :], t2[0:1, :])
                nc.sync.dma_start(fo_d[b:b + 1, :], t2[1:2, :])
    nc.compile()
    return nc


# ---------------------------------------------------------------- entry
_CACHE = {}


def _tree_np(x):
    if isinstance(x, dict):
        return {k: _tree_np(v) for k, v in x.items()}
    if isinstance(x, (list, tuple)):
        return type(x)(_tree_np(v) for v in x)
    return np.asarray(x)


LAST_EXEC_NS = None


def kernel(fps_point, point_coord, params):
    global LAST_EXEC_NS
    from concourse.bass_utils import run_bass_kernel_spmd

    fps_point = np.asarray(fps_point, np.float32)
    point_coord = np.asarray(point_coord, np.float32)
    params = _tree_np(params)

    if "nc" not in _CACHE:
        _CACHE["nc"] = build_program()
    nc = _CACHE["nc"]

    bias_d, bias_g, scal, trans = _host_precompute(fps_point, point_coord, params)
    pcs, shared = _pack_device_inputs(point_coord, params, bias_d, bias_g, scal, trans)
    in_maps = [dict(shared, pc=pcs[c]) for c in range(NCORES)]

    res = run_bass_kernel_spmd(nc, in_maps, list(range(NCORES)))
    LAST_EXEC_NS = getattr(res, "exec_time_ns", None)

    Gs = np.empty((B, NQ, T), np.float32)
    fine = np.empty((B, NQ, 1), np.float32)
    for c in range(NCORES):
        r = res.results[c]
        Gs[:, c * QC:(c + 1) * QC, :] = r["gs"].transpose(0, 2, 1)
        fine[:, c * QC:(c + 1) * QC, 0] = r["fo"]
    out = Gs.max(axis=2, keepdims=True)
    return Gs, out, fine


# revision 20
# speedup vs baseline: 1.0123x; 1.0123x over previous
"""Trainium2 Bass kernel for nn_BAE_k_deform_k_bae_corse2fine_affine.

Strategy:
  - Host (numpy, fp32): PointNet++ encoder (FPS / ball-query / small SA MLPs,
    ~2 GFLOP total), the z-embedding chains, pose MLP, and folding of all
    z-dependent biases.  These are tiny and/or inherently sequential.
  - Device (8 NeuronCores): the per-query-point MLP stacks (deform MLP,
    per-part BAE MLP, coarse2fine MLP; ~248 GFLOP), sharded over the
    query-point axis (1024 points per core per batch).  Weights replicated.
  - All first-layer (K=3) biases are folded into the matmul via an extra
    ones-row on the moving operand; mid-layer biases ride the activation op.
"""

import numpy as np
from contextlib import ExitStack

# ---------------------------------------------------------------- constants
T, PD, ZD, GF = 4, 64, 256, 256
B, NQ, NCORES = 4, 8192, 8
QC = NQ // NCORES            # query points per core (per batch)
NT = 512                     # device point-tile (matmul moving free dim)
NTILES = QC // NT
BN_SCALE = np.float32(1.0 / np.sqrt(1.0 + 1e-5))
SA_CFG = [
    (1024, 0.1, 32, 6, [32, 32, 64]),
    (256, 0.2, 32, 67, [64, 64, 128]),
    (64, 0.4, 32, 131, [128, 128, 256]),
    (16, 0.8, 32, 259, [256, 256, ZD]),
]

MM_DTYPE = "f32r"            # 'f32r' (fast fp32 PE mode) or 'f32'

# --- schedule tuning knobs (read by build_program) ---
CFG = {
    "pair_dl1": True,    # paired evac for deform L1
    "pair_bl1": True,    # paired evac for bae L1
    "pair_fl1": False,    # paired evac for fine L1
    "ps2_bufs": 2,       # [128,1024] psum pool (2 banks each)
    "ps_bufs": 3,        # [128,512] psum pool
    "pss_bufs": 1,       # small psum pool
    "g1f1_bufs": 12,
    "pf2_bufs": 9,
    "dh1_bufs": 3,
    "dh2_bufs": 4,
    "xg_bufs": 5,
    "gs_bufs": 2,
    "deform_first": True,
    "bae_sync": 2,       # 0: chained parts; 2: pair-synced; 4: fully layer-synced
}

# wd23 region columns: dw2[i] @ i*512 (k*256+m*128), dw3[i] @ 2048+i*8 (k*4)
WD23_COLS = 2080
# wb2: bw2[i] @ i*2048 (k*256+m*128) [bf16]; wb3: bw3[i] @ i*2 (k) [f32r]
WB2_COLS = 8192
WB3_COLS = 8
WF1_COLS = 8192              # k*1024 + m*128
WF2_COLS = 8192
WF3_COLS = 8                 # k
WTS3_COLS = 40 * 128         # per-b: 40 tiles of [4,128] side by side (base partition 0)
WF1C_COLS = 8 * 128          # fine L1 coord tiles (batch-independent)
BIAS_COLS = 41

# bias column map
def _bc_db2(i, m): return i * 2 + m          # deform L2 bias   [128]
def _bc_db3(i): return 8 + i                 # deform L3 bias   [4] (row3=0)
def _bc_bb2(i, m): return 12 + i * 2 + m     # bae L2 bias      [128]
def _bc_bb3(i): return 20 + i                # bae L3 bias      [1]
def _bc_fb2(m): return 24 + m                # fine L2 bias     [128]
def _bc_fb3(): return 32                     # fine L3 bias     [1]
def _bc_scal(i): return 33 + i               # affine scale     [4] (row3=0)
def _bc_trans(i): return 37 + i              # affine trans     [4] (row3=0)

def _wts3_tile_dw1(i, m): return i * 2 + m          # per-b tile index 0..7
def _wts3_tile_bw1(i, m): return 8 + i * 8 + m       # per-b tile index 8..39


# ---------------------------------------------------------------- host math
def _gather(a, idx):
    # a [B,N,C], idx [B,...] -> [B,...,C]
    return a[np.arange(a.shape[0])[:, None], idx.reshape(idx.shape[0], -1)].reshape(
        idx.shape + (a.shape[-1],)
    )


def _fps(xyz, npoint):
    b, n, _ = xyz.shape
    dist = np.full((b, n), 1e10, np.float32)
    far = np.zeros((b,), np.int32)
    idx = np.empty((npoint, b), np.int32)
    ar = np.arange(b)
    for s in range(npoint):
        idx[s] = far
        c = xyz[ar, far][:, None, :]
        d = ((xyz - c) ** 2).sum(-1)
        dist = np.minimum(dist, d)
        far = np.argmax(dist, -1).astype(np.int32)
    return idx.T


def _query_ball(radius, nsample, xyz, new_xyz):
    b, n, _ = xyz.shape
    s = new_xyz.shape[1]
    out = np.empty((b, s, nsample), np.int32)
    r2 = np.float32(radius) * np.float32(radius)
    arn = np.arange(n, dtype=np.int32)
    chunk = max(1, (1 << 22) // n)
    for c0 in range(0, s, chunk):
        c1 = min(s, c0 + chunk)
        sqr = ((new_xyz[:, c0:c1, None, :] - xyz[:, None, :, :]) ** 2).sum(-1)
        idx = np.where(sqr > r2, np.int32(n), arn[None, None, :])
        idx = np.sort(idx, axis=-1)[:, :, :nsample]
        first = idx[:, :, :1]
        out[:, c0:c1] = np.where(idx == n, first, idx)
    return out


def _sa(xyz, pts, npoint, radius, nsample, convs):
    fidx = _fps(xyz, npoint)
    new_xyz = _gather(xyz, fidx)
    gidx = _query_ball(radius, nsample, xyz, new_xyz)
    x = np.concatenate(
        [_gather(xyz, gidx) - new_xyz[:, :, None, :], _gather(pts, gidx)], -1
    )
    for w, b_ in convs:
        x = np.maximum((x @ w + b_) * BN_SCALE, 0.0).astype(np.float32)
    return new_xyz, x.max(axis=2)


def _host_precompute(fps_point, point_coord, params):
    enc, de, gen = params["enc"], params["deform"], params["gen"]
    xyz = pts = np.asarray(fps_point, np.float32)
    for i, (npoint, r, k, _, _) in enumerate(SA_CFG):
        convs = [(np.asarray(w, np.float32), np.asarray(b_, np.float32))
                 for (w, b_) in enc["sa%d" % i]]
        xyz, pts = _sa(xyz, pts, npoint, r, k, convs)
    z = np.einsum("bnc,n->bc", pts, np.asarray(enc["conv0_w"], np.float32)) + np.asarray(
        enc["conv0_b"], np.float32
    )  # [B, ZD]

    def chain(z0, layers):
        e = z0
        for w, b_ in layers:
            e = e @ np.asarray(w, np.float32) + np.asarray(b_, np.float32)
        return e

    ez_d = chain(z, de["embed1"])          # [B, T*PD]
    ez_g = chain(z, gen["embed1"])         # [B, T*PD]
    pcodes = np.asarray(de["part_codes"], np.float32)          # [T, PD]
    ep = pcodes @ np.asarray(gen["embed2_w"], np.float32) + np.asarray(
        gen["embed2_b"], np.float32
    )                                                           # [T, PD]
    (pw1, pb1), (pw2, pb2), (pw3, pb3) = [
        (np.asarray(w, np.float32), np.asarray(b_, np.float32)) for (w, b_) in gen["pose"]
    ]
    pp = np.maximum(ep @ pw1 + pb1, 0.0)
    pp = np.maximum(pp @ pw2 + pb2, 0.0)
    pp = pp @ pw3 + pb3                                         # [T, 6]
    trans = np.tanh(pp[:, :3]).astype(np.float32)               # [T, 3]
    scal = (1.0 / (1.0 + np.exp(-pp[:, 3:]))).astype(np.float32)

    e3w = np.asarray(gen["embed3_w"], np.float32)
    e3b = np.asarray(gen["embed3_b"], np.float32)
    bias_d = np.empty((B, T, 256), np.float32)
    bias_g = np.empty((B, T, GF * 4), np.float32)
    for i in range(T):
        zdef = ez_d[:, i * PD:(i + 1) * PD] + pcodes[i]         # [B, PD]
        w1, b1 = de["w1"][i], de["b1"][i]
        bias_d[:, i] = zdef @ np.asarray(w1, np.float32)[3:] + np.asarray(b1, np.float32)
        zf = (ez_g[:, i * PD:(i + 1) * PD] + ep[i]) @ e3w + e3b  # [B, PD]
        bw1, bb1 = gen["bae_w1"][i], gen["bae_b1"][i]
        bias_g[:, i] = zf @ np.asarray(bw1, np.float32)[3:] + np.asarray(bb1, np.float32)
    return bias_d, bias_g, scal, trans


def _pack_ktiles(w):
    k, m = w.shape
    nk = k // 128
    return np.ascontiguousarray(
        w.reshape(nk, 128, m).transpose(1, 0, 2).reshape(128, nk * m)
    )


def _pack_device_inputs(point_coord, params, bias_d, bias_g, scal, trans):
    de, gen = params["deform"], params["gen"]
    f32 = np.float32

    import ml_dtypes
    bf16 = ml_dtypes.bfloat16
    wd23 = np.zeros((128, WD23_COLS), f32)
    wb2 = np.zeros((128, WB2_COLS), bf16)
    wb3 = np.zeros((128, WB3_COLS), f32)
    for i in range(T):
        wd23[:, i * 512:(i + 1) * 512] = _pack_ktiles(np.asarray(de["w2"][i], f32))
        w3 = np.zeros((256, 4), f32)
        w3[:, :3] = np.asarray(de["w3"][i], f32)
        wd23[:, 2048 + i * 8:2048 + (i + 1) * 8] = _pack_ktiles(w3)
        wb2[:, i * 2048:(i + 1) * 2048] = _pack_ktiles(
            np.asarray(gen["bae_w2"][i], f32)).astype(bf16)
        wb3[:, i * 2:(i + 1) * 2] = _pack_ktiles(np.asarray(gen["bae_w3"][i], f32))
    fw1 = np.asarray(gen["fine_w1"], f32)          # [1027, 1024]
    wf1 = _pack_ktiles(fw1[3:])
    wf2 = _pack_ktiles(np.asarray(gen["fine_w2"], f32)).astype(bf16)
    wf3 = _pack_ktiles(np.asarray(gen["fine_w3"], f32))

    wts3 = np.zeros((B, 4, WTS3_COLS), f32)

    def put3(b, tidx, rows3, brow):
        c = 128 * tidx
        wts3[b, 0:3, c:c + 128] = rows3
        wts3[b, 3, c:c + 128] = brow

    for b in range(B):
        for i in range(T):
            w1 = np.asarray(de["w1"][i], f32)
            for m in range(2):
                put3(b, _wts3_tile_dw1(i, m), w1[0:3, m * 128:(m + 1) * 128],
                     bias_d[b, i, m * 128:(m + 1) * 128])
            bw1 = np.asarray(gen["bae_w1"][i], f32)
            for m in range(8):
                put3(b, _wts3_tile_bw1(i, m), bw1[0:3, m * 128:(m + 1) * 128],
                     bias_g[b, i, m * 128:(m + 1) * 128])
    wf1c = np.zeros((4, WF1C_COLS), f32)
    fb1 = np.asarray(gen["fine_b1"], f32)
    for m in range(8):
        wf1c[0:3, m * 128:(m + 1) * 128] = fw1[0:3, m * 128:(m + 1) * 128]
        wf1c[3, m * 128:(m + 1) * 128] = fb1[m * 128:(m + 1) * 128]

    bias = np.zeros((128, BIAS_COLS), f32)
    for i in range(T):
        db2 = np.asarray(de["b2"][i], f32)
        bb2 = np.asarray(gen["bae_b2"][i], f32)
        for m in range(2):
            bias[:, _bc_db2(i, m)] = db2[m * 128:(m + 1) * 128]
            bias[:, _bc_bb2(i, m)] = bb2[m * 128:(m + 1) * 128]
        bias[0:3, _bc_db3(i)] = np.asarray(de["b3"][i], f32)
        bias[0, _bc_bb3(i)] = np.asarray(gen["bae_b3"][i], f32)[0]
        bias[0:3, _bc_scal(i)] = scal[i]
        bias[0:3, _bc_trans(i)] = trans[i]
    fb2 = np.asarray(gen["fine_b2"], f32)
    for m in range(8):
        bias[:, _bc_fb2(m)] = fb2[m * 128:(m + 1) * 128]
    bias[0, _bc_fb3()] = np.asarray(gen["fine_b3"], f32)[0]

    # per-core transposed coords with ones row
    pc_full = np.ones((B, 4, NQ), f32)
    pc_full[:, 0:3, :] = np.asarray(point_coord, f32).transpose(0, 2, 1)
    pcs = [np.ascontiguousarray(pc_full[:, :, c * QC:(c + 1) * QC]) for c in range(NCORES)]

    shared = {"wd23": wd23, "wb2": wb2, "wb3": wb3, "wf1": wf1, "wf2": wf2,
              "wf3": wf3, "wts3": wts3, "wf1c": wf1c, "bias": bias}
    return pcs, shared


# ---------------------------------------------------------------- device IR
def build_program():
    import concourse.bass as bass
    import concourse.mybir as mybir
    import concourse.tile as tile
    from concourse import bacc
    from contextlib import ExitStack

    f32 = mybir.dt.float32
    AF = mybir.ActivationFunctionType
    OP = mybir.AluOpType
    # float32r tiles are produced rounded, letting the PE run its fast
    # single-pass fp32 mode (1 cycle/row at N>=256).
    fr = mybir.dt.float32r if MM_DTYPE == "f32r" else f32

    nc = bacc.Bacc("TRN2", target_bir_lowering=False, debug=False)
    pc_d = nc.dram_tensor("pc", [B, 4, QC], fr, kind="ExternalInput").ap()
    wd23_d = nc.dram_tensor("wd23", [128, WD23_COLS], fr, kind="ExternalInput").ap()
    bf = mybir.dt.bfloat16
    wb2_d = nc.dram_tensor("wb2", [128, WB2_COLS], bf, kind="ExternalInput").ap()
    wb3_d = nc.dram_tensor("wb3", [128, WB3_COLS], fr, kind="ExternalInput").ap()
    wf1_d = nc.dram_tensor("wf1", [128, WF1_COLS], fr, kind="ExternalInput").ap()
    wf2_d = nc.dram_tensor("wf2", [128, WF2_COLS], bf, kind="ExternalInput").ap()
    wf3_d = nc.dram_tensor("wf3", [128, WF3_COLS], fr, kind="ExternalInput").ap()
    wts3_d = nc.dram_tensor("wts3", [B, 4, WTS3_COLS], fr, kind="ExternalInput").ap()
    wf1c_d = nc.dram_tensor("wf1c", [4, WF1C_COLS], fr, kind="ExternalInput").ap()
    bias_d_ = nc.dram_tensor("bias", [128, BIAS_COLS], f32, kind="ExternalInput").ap()
    gs_d = nc.dram_tensor("gs", [B, T, QC], f32, kind="ExternalOutput").ap()
    fo_d = nc.dram_tensor("fo", [B, QC], f32, kind="ExternalOutput").ap()

    N2 = 2 * NT

    with tile.TileContext(nc) as tc:
        with ExitStack() as ctx:
            wp = ctx.enter_context(tc.tile_pool(name="weights", bufs=1))
            xdp = ctx.enter_context(tc.tile_pool(name="xd", bufs=2))
            affp = ctx.enter_context(tc.tile_pool(name="aff", bufs=2))
            xgp = ctx.enter_context(tc.tile_pool(name="xg", bufs=CFG["xg_bufs"]))
            dh1p = ctx.enter_context(tc.tile_pool(name="dh1", bufs=CFG["dh1_bufs"]))
            dh2p = ctx.enter_context(tc.tile_pool(name="dh2", bufs=CFG["dh2_bufs"]))
            g1f1p = ctx.enter_context(tc.tile_pool(name="g1f1", bufs=CFG["g1f1_bufs"]))
            pf2p = ctx.enter_context(tc.tile_pool(name="pf2", bufs=CFG["pf2_bufs"]))
            gsp = ctx.enter_context(tc.tile_pool(name="gsst", bufs=CFG["gs_bufs"]))
            fsbp = ctx.enter_context(tc.tile_pool(name="fsb", bufs=2))
            psp2 = ctx.enter_context(tc.tile_pool(name="ps2", bufs=CFG["ps2_bufs"], space="PSUM"))
            psp = ctx.enter_context(tc.tile_pool(name="ps", bufs=CFG["ps_bufs"], space="PSUM"))
            pssp = ctx.enter_context(tc.tile_pool(name="pss", bufs=CFG["pss_bufs"], space="PSUM"))

            bias = wp.tile([128, BIAS_COLS], f32, tag="bias")
            nc.sync.dma_start(bias[:], bias_d_[:])
            # big weight blocks are DMA'd lazily (below) so the first combo's
            # staging/coord loads aren't queued behind ~7MB of weights
            wd23 = wp.tile([128, WD23_COLS], fr, tag="wd23")
            wb2 = wp.tile([128, WB2_COLS], bf, tag="wb2")
            wb3 = wp.tile([128, WB3_COLS], fr, tag="wb3")
            wf1 = wp.tile([128, WF1_COLS], fr, tag="wf1")
            wf2 = wp.tile([128, WF2_COLS], bf, tag="wf2")
            wf3 = wp.tile([128, WF3_COLS], fr, tag="wf3")
            wf1c = wp.tile([4, WF1C_COLS], fr, tag="wf1c")
            cm1 = wp.tile([4, NT], f32, tag="cm1")
            nc.gpsimd.memset(cm1[:], -1.0)
            cp1 = wp.tile([4, NT], f32, tag="cp1")
            nc.gpsimd.memset(cp1[:], 1.0)

            def w3ap(stage, tidx):
                return stage[0:4, 128 * tidx:128 * tidx + 128]

            def bcol(c, rows=128):
                return bias[0:rows, c:c + 1]

            def prelu(dst, ps, bias_ap=0.0):
                nc.scalar.activation(dst, ps, AF.Prelu, bias=bias_ap, alpha=0.01)

            for b in range(B):
                wts3 = wp.tile([4, WTS3_COLS], fr, tag="wts3", name="wts3st", bufs=1)
                nc.sync.dma_start(wts3[:], wts3_d[b])
                for t in range(NTILES):
                    sl = slice(t * NT, (t + 1) * NT)
                    xd = xdp.tile([4, NT], fr, tag="xd")
                    nc.sync.dma_start(xd[:], pc_d[b, :, sl])
                    if b == 0 and t == 0:
                        nc.sync.dma_start(wd23[:], wd23_d[:])
                        nc.sync.dma_start(wb2[:], wb2_d[:])
                        nc.sync.dma_start(wb3[:], wb3_d[:])
                    pfe = [pf2p.tile([128, NT], fr, tag="pf2", name="pfe")
                           for _ in range(8)]
                    def deform_chain(i):
                        # deform L1: K=4 (coords+ones), M=256
                        h1 = dh1p.tile([128, N2], fr, tag="dh1", name="dh1t")
                        if CFG["pair_dl1"]:
                            pp = psp2.tile([128, N2], f32, tag="ps2", name="pdl1")
                            for m in range(2):
                                nc.tensor.matmul(pp[:, m * NT:(m + 1) * NT],
                                                 w3ap(wts3, _wts3_tile_dw1(i, m)),
                                                 xd[:], start=True, stop=True)
                            prelu(h1[:], pp[:])
                        else:
                            for m in range(2):
                                ps = psp.tile([128, NT], f32, tag="ps", name="pdl1")
                                nc.tensor.matmul(ps[:], w3ap(wts3, _wts3_tile_dw1(i, m)),
                                                 xd[:], start=True, stop=True)
                                prelu(h1[:, m * NT:(m + 1) * NT], ps[:])
                        # deform L2: K=256, M=256
                        h2 = []
                        for m in range(2):
                            ps = psp.tile([128, NT], f32, tag="ps", name="pdl2")
                            for k in range(2):
                                w = wd23[:, i * 512 + k * 256 + m * 128:
                                         i * 512 + k * 256 + m * 128 + 128]
                                nc.tensor.matmul(ps[:], w, h1[:, k * NT:(k + 1) * NT],
                                                 start=(k == 0), stop=(k == 1))
                            h = dh2p.tile([128, NT], fr, tag="dh2", name="dh2t")
                            prelu(h[:], ps[:], bcol(_bc_db2(i, m)))
                            h2.append(h)
                        # deform L3 (M=4 padded; row3 stays 0) + tanh + affine
                        ps3 = pssp.tile([4, NT], f32, tag="pss", name="pdl3")
                        for k in range(2):
                            w = wd23[:, 2048 + i * 8 + k * 4:2048 + i * 8 + k * 4 + 4]
                            nc.tensor.matmul(ps3[:], w, h2[k][:],
                                             start=(k == 0), stop=(k == 1))
                        t3 = affp.tile([4, NT], f32, tag="aff", name="t3t")
                        nc.scalar.activation(t3[:], ps3[:], AF.Tanh,
                                             bias=bcol(_bc_db3(i), 4))
                        nc.scalar.activation(t3[:], t3[:], AF.Identity,
                                             bias=bcol(_bc_trans(i), 4),
                                             scale=bcol(_bc_scal(i), 4))
                        nc.vector.tensor_tensor(t3[:], t3[:], cm1[:], OP.max)
                        nc.vector.tensor_tensor(t3[:], t3[:], cp1[:], OP.min)
                        xg = xgp.tile([4, NT], fr, tag="xg")
                        nc.vector.tensor_tensor(xg[:], t3[:], xd[:], OP.add)
                        return xg

                    def bae_l1(i, xg):
                        # bae L1: K=4, M=1024 as 4 [128,1024] outputs
                        g1 = []
                        for mp in range(4):
                            g = g1f1p.tile([128, N2], bf, tag="g1f1", name="g1t")
                            if CFG["pair_bl1"]:
                                pp = psp2.tile([128, N2], f32, tag="ps2", name="pbl1")
                                for h in range(2):
                                    nc.tensor.matmul(
                                        pp[:, h * NT:(h + 1) * NT],
                                        w3ap(wts3, _wts3_tile_bw1(i, 2 * mp + h)),
                                        xg[:], start=True, stop=True)
                                prelu(g[:], pp[:])
                            else:
                                for h in range(2):
                                    ps = psp.tile([128, NT], f32, tag="ps", name="pbl1")
                                    nc.tensor.matmul(
                                        ps[:], w3ap(wts3, _wts3_tile_bw1(i, 2 * mp + h)),
                                        xg[:], start=True, stop=True)
                                    prelu(g[:, h * NT:(h + 1) * NT], ps[:])
                            g1.append(g)
                        return g1

                    def bae_l23(i, g1):
                        # bae L2: K=1024, M=256
                        for m in range(2):
                            ps = psp.tile([128, NT], f32, tag="ps", name="pbl2")
                            for k in range(8):
                                w = wb2[:, i * 2048 + k * 256 + m * 128:
                                        i * 2048 + k * 256 + m * 128 + 128]
                                nc.tensor.matmul(ps[:], w,
                                                 g1[k // 2][:, (k % 2) * NT:
                                                            (k % 2) * NT + NT],
                                                 start=(k == 0), stop=(k == 7))
                            prelu(pfe[i * 2 + m][:], ps[:], bcol(_bc_bb2(i, m)))
                        # bae L3 logit + sigmoid
                        psg = pssp.tile([1, NT], f32, tag="pss", name="pbl3")
                        for k in range(2):
                            w = wb3[:, i * 2 + k:i * 2 + k + 1]
                            nc.tensor.matmul(psg[:], w, pfe[i * 2 + k][:],
                                             start=(k == 0), stop=(k == 1))
                        gs_sb = gsp.tile([1, NT], f32, tag="gs", name="gs_sb")
                        nc.scalar.activation(gs_sb[:], psg[:], AF.Sigmoid,
                                             bias=bcol(_bc_bb3(i), 1))
                        nc.sync.dma_start(gs_d[b, i:i + 1, sl], gs_sb[:])

                    def bae_chain(i, xg):
                        bae_l23(i, bae_l1(i, xg))

                    if b == 0 and t == 0:
                        nc.sync.dma_start(wf1c[:], wf1c_d[:])
                        nc.sync.dma_start(wf1[:], wf1_d[:])
                        nc.sync.dma_start(wf2[:], wf2_d[:])
                        nc.sync.dma_start(wf3[:], wf3_d[:])
                    if CFG["deform_first"]:
                        xgs = [deform_chain(i) for i in range(T)]
                        sync = CFG["bae_sync"]
                        if sync == 0:
                            for i in range(T):
                                bae_chain(i, xgs[i])
                        else:
                            for p0 in range(0, T, sync):
                                grp = range(p0, p0 + sync)
                                g1s = {i: bae_l1(i, xgs[i]) for i in grp}
                                for i in grp:
                                    bae_l23(i, g1s[i])
                    else:
                        for i in range(T):
                            bae_chain(i, deform_chain(i))
                    # fine L1: K=1027 (coords+ones fold + pfe), M=1024, 4 pairs
                    f1 = []
                    for mp in range(4):
                        g = g1f1p.tile([128, N2], bf, tag="g1f1", name="f1t")
                        if CFG["pair_fl1"]:
                            pp = psp2.tile([128, N2], f32, tag="ps2", name="pfl1")
                            for h in range(2):
                                m = 2 * mp + h
                                dst = pp[:, h * NT:(h + 1) * NT]
                                nc.tensor.matmul(dst, w3ap(wf1c, m), xd[:],
                                                 start=True, stop=False)
                                for k in range(8):
                                    w = wf1[:, k * 1024 + m * 128:k * 1024 + m * 128 + 128]
                                    nc.tensor.matmul(dst, w, pfe[k][:],
                                                     start=False, stop=(k == 7))
                            prelu(g[:], pp[:])
                        else:
                            for h in range(2):
                                m = 2 * mp + h
                                ps = psp.tile([128, NT], f32, tag="ps", name="pfl1")
                                nc.tensor.matmul(ps[:], w3ap(wf1c, m), xd[:],
                                                 start=True, stop=False)
                                for k in range(8):
                                    w = wf1[:, k * 1024 + m * 128:k * 1024 + m * 128 + 128]
                                    nc.tensor.matmul(ps[:], w, pfe[k][:],
                                                     start=False, stop=(k == 7))
                                prelu(g[:, h * NT:(h + 1) * NT], ps[:])
                        f1.append(g)
                    # fine L2: K=1024, M=1024
                    f2 = []
                    for m in range(8):
                        ps = psp.tile([128, NT], f32, tag="ps", name="pfl2")
                        for k in range(8):
                            w = wf2[:, k * 1024 + m * 128:k * 1024 + m * 128 + 128]
                            nc.tensor.matmul(ps[:], w,
                                             f1[k // 2][:, (k % 2) * NT:(k % 2) * NT + NT],
                                             start=(k == 0), stop=(k == 7))
                        g = pf2p.tile([128, NT], fr, tag="pf2", name="f2t")
                        prelu(g[:], ps[:], bcol(_bc_fb2(m)))
                        f2.append(g)
                    # fine L3 + sigmoid
                    psf = pssp.tile([1, NT], f32, tag="pss", name="pfl3")
                    for k in range(8):
                        nc.tensor.matmul(psf[:], wf3[:, k:k + 1], f2[k][:],
                                         start=(k == 0), stop=(k == 7))
                    fsb = fsbp.tile([1, NT], f32, tag="fsb")
                    nc.scalar.activation(fsb[:], psf[:], AF.Sigmoid,
                                         bias=bcol(_bc_fb3(), 1))
                    nc.sync.dma_start(fo_d[b:b + 1, sl], fsb[:])
    nc.compile()
    return nc


def build_noop_program():
    """Same I/O signature, trivial body — times the dispatch/transfer floor."""
    import concourse.mybir as mybir
    import concourse.tile as tile
    from concourse import bacc
    from contextlib import ExitStack

    f32 = mybir.dt.float32
    fr = mybir.dt.float32r if MM_DTYPE == "f32r" else f32
    nc = bacc.Bacc("TRN2", target_bir_lowering=False, debug=False)
    pc_d = nc.dram_tensor("pc", [B, 4, QC], fr, kind="ExternalInput").ap()
    nc.dram_tensor("wd23", [128, WD23_COLS], fr, kind="ExternalInput")
    nc.dram_tensor("wb23", [128, WB23_COLS], fr, kind="ExternalInput")
    nc.dram_tensor("wf1", [128, WF1_COLS], fr, kind="ExternalInput")
    nc.dram_tensor("wf2", [128, WF2_COLS], fr, kind="ExternalInput")
    nc.dram_tensor("wf3", [128, WF3_COLS], fr, kind="ExternalInput")
    nc.dram_tensor("wts3", [B, 4, WTS3_COLS], fr, kind="ExternalInput")
    nc.dram_tensor("wf1c", [4, WF1C_COLS], fr, kind="ExternalInput")
    nc.dram_tensor("bias", [128, BIAS_COLS], f32, kind="ExternalInput")
    gs_d = nc.dram_tensor("gs", [B, T, QC], f32, kind="ExternalOutput").ap()
    fo_d = nc.dram_tensor("fo", [B, QC], f32, kind="ExternalOutput").ap()
    with tile.TileContext(nc) as tc:
        with ExitStack() as ctx:
            p = ctx.enter_context(tc.tile_pool(name="p", bufs=1))
            t = p.tile([4, QC], fr, tag="t")
            nc.sync.dma_start(t[:], pc_d[0])
            t2 = p.tile([4, QC], f32, tag="t2")
            nc.vector.tensor_copy(t2[:], t[:])
            for b in range(B):
                for i in range(T):
                    nc.sync.dma_start(gs_d[b, i:i + 1, :], t2[0:1, :])
                nc.sync.dma_start(fo_d[b:b + 1, :], t2[1:2, :])
    nc.compile()
    return nc


# ---------------------------------------------------------------- entry
_CACHE = {}


def _tree_np(x):
    if isinstance(x, dict):
        return {k: _tree_np(v) for k, v in x.items()}
    if isinstance(x, (list, tuple)):
        return type(x)(_tree_np(v) for v in x)
    return np.asarray(x)


LAST_EXEC_NS = None


def kernel(fps_point, point_coord, params):
    global LAST_EXEC_NS
    from concourse.bass_utils import run_bass_kernel_spmd

    fps_point = np.asarray(fps_point, np.float32)
    point_coord = np.asarray(point_coord, np.float32)
    params = _tree_np(params)

    if "nc" not in _CACHE:
        _CACHE["nc"] = build_program()
    nc = _CACHE["nc"]

    bias_d, bias_g, scal, trans = _host_precompute(fps_point, point_coord, params)
    pcs, shared = _pack_device_inputs(point_coord, params, bias_d, bias_g, scal, trans)
    in_maps = [dict(shared, pc=pcs[c]) for c in range(NCORES)]

    res = run_bass_kernel_spmd(nc, in_maps, list(range(NCORES)))
    LAST_EXEC_NS = getattr(res, "exec_time_ns", None)

    Gs = np.empty((B, NQ, T), np.float32)
    fine = np.empty((B, NQ, 1), np.float32)
    for c in range(NCORES):
        r = res.results[c]
        Gs[:, c * QC:(c + 1) * QC, :] = r["gs"].transpose(0, 2, 1)
        fine[:, c * QC:(c + 1) * QC, 0] = r["fo"]
    out = Gs.max(axis=2, keepdims=True)
    return Gs, out, fine
